# revision 48
# baseline (speedup 1.0000x reference)
"""Trainium2 Bass kernel for nn_EventDecoder (segment-softmax aggregation + linear).

Computation (per plane p in {u, v, y}):
    x = m_p.reshape(N, C*D)                      # [N, 320]
    e = exp(t_p * x)                             # shift-free segment softmax
    den[s, f] = sum_{i: batch_p[i]=s} e[i, f]
    num[s, f] = sum_{i: batch_p[i]=s} e[i, f] * x[i, f]
    feat_p = num / den                           # [B, 320]
out = concat(feat_u, feat_v, feat_y) @ W.T + b   # [B, 3]

Sharding: batch indices are sorted, so segments are contiguous node runs.
Core k owns segments [8k, 8k+8) of all three planes -> no collectives.

Final datapath (bf16 end-to-end; tolerance 2e-2 leaves ample margin):
  - x is downcast to bf16 on the host (halves HBM traffic vs f32); the
    one-hot rows are precomputed on the host too and ride in the same
    chunk DMA (cols [nt*320, nt*328) of each chunk slot).
  - Per 16-tile chunk, exp splits between engines (error cancels in
    num/den): most tiles on ACT (spline exp, bf16 in/out); the last n_s
    tiles on the DVE via Schraudolph (one 4x-mode tensor_scalar
    z = x*c1 + c2 -> int16 whose bits ARE bf16(2^z')).
  - e*x runs as bf16 tensor_tensor (2x DVE mode).  A GpSimd e*x offload
    was measured a net loss (shared POOL SBUF port slows DVE ~35%).
  - Segment sums are one-hot matmuls in bf16.  The PE instruction stream
    is the kernel's wall (~136 ns per 320-col matmul, serial: full-K
    LDWEIGHTS cannot overlap in-flight matmuls, so tile_position column
    groups give no concurrency here).  den and num share each tile's
    one-hot: the num matmul sets ldweights=False to reuse the loaded
    weights, halving LDWEIGHTS count; den/num accumulate in separate
    PSUM banks at partitions 0-7 (no cross-partition fixup needed).
  - Plane 0 starts with 4- and 8-tile chunks so compute ramps while DMA
    fills; x DMAs issue from the GpSimd queue (fastest SWDGE path).
  - Per-plane tails (1/den via reciprocal straight from PSUM — segments
    are provably non-empty, feats = num*recip, the plane's share of the
    3x960 linear) run mid-stream a few chunks after each plane's matmuls
    stop (pc-ordered completion makes the psum safe to read); the u+v
    partials and bias fold mid-stream too, so only plane y's tail and a
    single add remain after the last matmul.

Flow control: one semaphore per x-slot DMA; s_mm (PE finished chunk) is
the single slot-free signal (PE is the last consumer of every buffer);
ACT increments s_e per half chunk so DVE/PE start early.  Dependent DVE
ops (schraudolph write -> ex read) are separated by unrelated TTs.  PE
drains before the final tail reads PSUM.  GpSimd nop().then_inc is
avoided (its semaphore update does not honor a preceding tensor op's
write drain - caused stale reads).
"""

import sys

sys.path.insert(0, "/opt/trn_rl_repo")

import numpy as np
import ml_dtypes

BF16NP = ml_dtypes.bfloat16

N_CORES = 8
B = 64
SEG_PER_CORE = B // N_CORES          # 8 local segments per core
NSEG = SEG_PER_CORE
F = 320                              # C*D
FO = F + NSEG                        # x + onehot columns per tile
E_OUT = 3
CHUNK = 2048                         # nodes per full DMA chunk
TPC = CHUNK // 128                   # 16 node-tiles per full chunk
XFD = TPC * FO                       # bf16 per partition per full chunk slot
NBUF_X = 7                           # x chunk buffers
NSLOT = 5                            # e/ex chunk slots
PAD_SEG = NSEG                       # out-of-range id -> one-hot all zero
LN2 = float(np.log(2.0))
# per-chunk tile split (cycled by h%4): GpSimd-mult / DVE-schraudolph-exp
# (GpSimd e*x offload measured as a net loss: the shared POOL SBUF port
# slows concurrent DVE streaming ~35%, exceeding the offloaded work)
N_G = (0, 0, 0, 0)
N_S = (3, 4, 3, 4)

LAST_EXEC_TIME_NS = None

_prog_cache = {}


def _install_profile_shim():
    """Register the NTFF profile hook missing from this image so
    run_bass_kernel_spmd(trace=...) can report neuron-profile exec time."""
    import types
    import os

    if "antenv.axon_hooks" not in sys.modules:
        import antenv  # noqa: F401  (stub package; must exist)

        mod = types.ModuleType("antenv.axon_hooks")
        mod._hook = None
        mod.set_axon_ntff_profile_hook = lambda h: setattr(mod, "_hook", h)
        mod.get_axon_ntff_profile_hook = lambda: mod._hook
        sys.modules["antenv.axon_hooks"] = mod
    try:
        if "/root/.axon_site" not in sys.path:
            sys.path.insert(0, "/root/.axon_site")
        from trn_agent_boot.trn_boot import _ntff_profile_via_ctypes

        so_path = "/opt/axon/libaxon_pjrt.so"
        if os.path.exists(so_path):
            sys.modules["antenv.axon_hooks"].set_axon_ntff_profile_hook(
                _ntff_profile_via_ctypes(so_path)
            )
    except Exception:
        pass
    try:
        import concourse.bass_utils as bu

        bu.upload_artifacts = lambda tmpdir: tmpdir
    except Exception:
        pass


def _plan(p_n):
    """Chunk-level schedule, identical on every core.  Plane 0 starts with
    small chunks so the compute pipeline fills while DMA ramps."""
    total_tiles = p_n // 128
    chunks = []
    for p in range(3):
        base_t = 0
        remaining = total_tiles
        ci = 0
        while remaining > 0:
            if p == 0 and ci == 0 and remaining > 12:
                nt = 4
            elif p == 0 and ci == 1 and remaining > 24:
                nt = 8
            else:
                nt = min(TPC, remaining)
            chunks.append(dict(plane=p, g0=base_t, ntiles=nt, ci=ci,
                               first=(ci == 0), last=(remaining <= TPC)))
            base_t += nt
            remaining -= nt
            ci += 1
    for h, ch in enumerate(chunks):
        ch["h"] = h
        ch["slot"] = h % NBUF_X
        ch["use"] = h // NBUF_X
        ch["eslot"] = h % NSLOT
        if ch["ntiles"] == TPC:
            ch["ng"] = N_G[h % 4]
            ch["ns"] = N_S[h % 4]
        else:
            ch["ng"] = 0
            ch["ns"] = 0
    return chunks, total_tiles


def _build_program(p_n, t_vals):
    import concourse.bass as bass
    import concourse.mybir as mybir
    from contextlib import ExitStack

    F32 = mybir.dt.float32
    BF16 = mybir.dt.bfloat16
    I16 = mybir.dt.int16
    AF = mybir.ActivationFunctionType
    ALU = mybir.AluOpType
    AX = mybir.AxisListType

    chunks, total_tiles = _plan(p_n)
    EFD = TPC * F                     # e/ex elements per partition per chunk

    nc = bass.Bass()
    xs_d = [nc.declare_dram_parameter(f"x{p}", [p_n, FO], BF16, isOutput=False)
            for p in range(3)]
    CWF = E_OUT * 3 * F + E_OUT       # [W rows | b]
    cwf_d = nc.declare_dram_parameter("constsW", [128, CWF], F32,
                                      isOutput=False)
    out_d = nc.declare_dram_parameter("out", [NSEG, E_OUT], F32, isOutput=True)

    wb_off = 0
    bb_off = E_OUT * 3 * F

    es = ExitStack()
    with es:
        xbuf = es.enter_context(nc.sbuf_tensor("xbuf", [128, XFD * NBUF_X], BF16))
        cwf = es.enter_context(nc.sbuf_tensor("cwf", [128, CWF], F32))
        ebuf = es.enter_context(nc.sbuf_tensor("ebuf", [128, EFD * NSLOT], BF16))
        exbuf = es.enter_context(nc.sbuf_tensor("exbuf", [128, EFD * NSLOT], BF16))
        featsb = es.enter_context(nc.sbuf_tensor("featsb", [128, 3 * F], F32))
        feats2 = es.enter_context(nc.sbuf_tensor("feats2", [128, 3 * F], F32))
        scratch = es.enter_context(nc.sbuf_tensor("scratch", [128, CWF - E_OUT], F32))
        redsb = es.enter_context(nc.sbuf_tensor("redsb", [128, 12], F32))
        outsb = es.enter_context(nc.sbuf_tensor("outsb", [128, E_OUT], F32))
        ps_den = [es.enter_context(nc.psum_tensor(f"psd{p}", [NSEG, 512], F32))
                  for p in range(3)]
        ps_num = [es.enter_context(nc.psum_tensor(f"psn{p}", [NSEG, 512], F32))
                  for p in range(3)]
        ps_warm = es.enter_context(nc.psum_tensor("ps_warm", [NSEG, 512], F32))
        s_cload = es.enter_context(nc.semaphore("s_cload"))
        s_loads = [es.enter_context(nc.semaphore(f"s_load{j}"))
                   for j in range(NBUF_X)]
        s_e = es.enter_context(nc.semaphore("s_e"))
        s_e1 = es.enter_context(nc.semaphore("s_e1"))
        s_ex = es.enter_context(nc.semaphore("s_ex"))
        s_exg = es.enter_context(nc.semaphore("s_exg"))
        s_mm = es.enter_context(nc.semaphore("s_mm"))
        s_pe_done = es.enter_context(nc.semaphore("s_pe_done"))
        s_fin1 = es.enter_context(nc.semaphore("s_fin1"))
        s_fin2 = es.enter_context(nc.semaphore("s_fin2"))
        s_fin = es.enter_context(nc.semaphore("s_fin"))
        s_out = es.enter_context(nc.semaphore("s_out"))
        block = es.enter_context(nc.Block())

        def x_sl(ch, t0, t1):
            """x columns of tiles [t0, t1) in chunk ch's slot."""
            base = ch["slot"] * XFD
            return xbuf[:, base + t0 * F:base + t1 * F]

        def oh_sl(ch, t):
            base = ch["slot"] * XFD + ch["ntiles"] * F
            return xbuf[:, base + t * NSEG:base + (t + 1) * NSEG]

        def e_sl(buf, ch, t0, t1):
            base = ch["eslot"] * EFD
            return buf[:, base + t0 * F:base + t1 * F]

        # h of each plane's last chunk; vector-stream insertion points for
        # the early per-plane tails (a few chunks into the next plane, so
        # the PE has surely finished and drained the plane's psum writes)
        last_h = {}
        for ch in chunks:
            last_h[ch["plane"]] = ch["h"]
        tailA_at = {last_h[0] + 5: 0, last_h[1] + 5: 1}

        @block.sync
        def _(sy):
            sy.dma_start(out=cwf[:, :], in_=cwf_d[:]).then_inc(s_cload, 16)
            sy.wait_ge(s_fin, 1)
            sy.dma_start(out=out_d[:], in_=outsb[0:NSEG, :]).then_inc(s_out, 16)
            sy.wait_ge(s_out, 16)

        @block.scalar
        def _(sc):
            # warm the exp table-set during the DMA ramp (~2.7us one-time)
            sc.activation(featsb[:, 0:8], cwf[:, 0:8], AF.Exp, scale=0.0)
            for ch in chunks:
                h, ng = ch["h"], ch["ng"]
                na_end = ch["ntiles"] - ch["ns"]     # ACT covers [0, na_end)
                sc.wait_ge(s_loads[ch["slot"]], 16 * (ch["use"] + 1))
                if h >= NSLOT:
                    sc.wait_ge(s_mm, h - NSLOT + 1)
                t = float(t_vals[ch["plane"]])
                half = min(na_end, ch["ntiles"] // 2)
                sc.activation(e_sl(ebuf, ch, 0, half), x_sl(ch, 0, half),
                              AF.Exp, scale=t).then_inc(s_e, 1)
                sc.activation(e_sl(ebuf, ch, half, na_end),
                              x_sl(ch, half, na_end),
                              AF.Exp, scale=t).then_inc(s_e, 1)

        @block.gpsimd
        def _(g):
            for ch in chunks:
                h = ch["h"]
                if h >= NBUF_X:
                    g.wait_ge(s_mm, h - NBUF_X + 1)
                nt = ch["ntiles"]
                base = ch["g0"] * 128
                # host stores each chunk as [128, nt*FO] C-order: DRAM row
                # p*nt + t holds FO elems -> per-partition contiguous reads
                src = xs_d[ch["plane"]][base:base + nt * 128, :] \
                    .rearrange("(p t) f -> p t f", p=128)
                dst = xbuf[:, ch["slot"] * XFD:ch["slot"] * XFD + nt * FO] \
                    .rearrange("p (t f) -> p t f", t=nt)
                g.dma_start(out=dst, in_=src).then_inc(s_loads[ch["slot"]], 16)

        @block.vector
        def _(v):

            def tail_plane(v, p, nchunks):
                # den -> 1/den, feats = num/den, plane-p share of the linear
                # (psum writes of plane p are drained: a later chunk's
                # matmuls completed, pc-ordered)
                if p == 2:
                    v.wait_ge(s_pe_done, 1)
                else:
                    v.wait_ge(s_mm, min(last_h[p] + 2, nchunks))
                lo = slice(0, NSEG)
                fe = featsb[lo, p * F:(p + 1) * F]
                # segments are provably non-empty (~4096 nodes each), so
                # den > 0 and the reciprocal can read PSUM directly
                v.reciprocal(fe, ps_den[p][lo, 0:F])
                v.drain()
                v.tensor_tensor(feats2[lo, p * F:(p + 1) * F],
                                ps_num[p][lo, 0:F], fe, ALU.mult)
                v.drain()
                for cc in range(E_OUT):
                    v.tensor_tensor(scratch[lo, (cc * 3 + p) * F:
                                            (cc * 3 + p + 1) * F],
                                    feats2[lo, p * F:(p + 1) * F],
                                    cwf[lo, wb_off + cc * 3 * F + p * F:
                                        wb_off + cc * 3 * F + (p + 1) * F],
                                    ALU.mult)
                v.drain()
                for cc in range(E_OUT):
                    v.reduce_sum(redsb[lo, p * E_OUT + cc:p * E_OUT + cc + 1],
                                 scratch[lo, (cc * 3 + p) * F:
                                         (cc * 3 + p + 1) * F],
                                 axis=AX.X)
                if p == 1:
                    v.drain()
                    v.tensor_tensor(redsb[lo, 9:12], redsb[lo, 0:E_OUT],
                                    redsb[lo, E_OUT:2 * E_OUT], ALU.add)
                    v.drain()
                    v.tensor_tensor(redsb[lo, 9:12], redsb[lo, 9:12],
                                    cwf[lo, bb_off:bb_off + E_OUT], ALU.add)

            nchunks = len(chunks)
            v.wait_ge(s_cload, 16)
            for ch in chunks:
                h = ch["h"]
                nt, ng, ns = ch["ntiles"], ch["ng"], ch["ns"]
                na_end = nt - ns
                if h in tailA_at:
                    tail_plane(v, tailA_at[h], nchunks)
                # s_e(2h+1) implies ACT(h) first half ran, which waited on
                # s_loads(h) and s_mm(h-NSLOT+1): slot guards are transitive.
                half = min(na_end, nt // 2)
                v.wait_ge(s_e, 2 * h + 1)
                if ns:
                    # schraudolph exp: int16(x*c1 + c2) bits are bf16 2^(..)
                    c1 = float(t_vals[ch["plane"]]) * 128.0 / LN2
                    v.tensor_scalar(
                        e_sl(ebuf, ch, na_end, nt).bitcast(I16),
                        x_sl(ch, na_end, nt),
                        c1, float(127 * 128), ALU.mult, ALU.add)
                # e*x first half (also spaces the schraudolph write from
                # the dependent read below)
                v.tensor_tensor(e_sl(exbuf, ch, 0, half),
                                e_sl(ebuf, ch, 0, half),
                                x_sl(ch, 0, half), ALU.mult).then_inc(s_ex, 1)
                v.wait_ge(s_e, 2 * h + 2)
                tt = v.tensor_tensor(e_sl(exbuf, ch, half, na_end),
                                     e_sl(ebuf, ch, half, na_end),
                                     x_sl(ch, half, na_end), ALU.mult)
                if ns:
                    tt = v.tensor_tensor(e_sl(exbuf, ch, na_end, nt),
                                         e_sl(ebuf, ch, na_end, nt),
                                         x_sl(ch, na_end, nt), ALU.mult)
                tt.then_inc(s_ex, 1)
            # ---- finalize: plane y tail + combine partial linears ----
            lo = slice(0, NSEG)
            tail_plane(v, 2, nchunks)
            v.drain()
            v.tensor_tensor(outsb[lo, 0:E_OUT], redsb[lo, 9:12],
                            redsb[lo, 2 * E_OUT:3 * E_OUT], ALU.add)
            v.drain()
            v.nop().then_inc(s_fin, 1)

        @block.tensor
        def _(te):
            for _ in range(45):
                te.matmul(ps_warm[0:NSEG, 0:F], xbuf[:, 0:NSEG],
                          xbuf[:, NSEG:NSEG + F], start=True, stop=True,
                          skip_group_check=True)
            gp_done = 0
            for ch in chunks:
                h = ch["h"]
                nt, ng = ch["ntiles"], ch["ng"]
                p = ch["plane"]
                half = min(nt - ch["ns"], nt // 2)
                te.wait_ge(s_ex, 2 * h + 1)
                for t in range(nt):
                    if t == half:
                        te.wait_ge(s_ex, 2 * h + 2)
                    lhsT = oh_sl(ch, t)
                    start = ch["first"] and t == 0
                    stop = ch["last"] and t == nt - 1
                    te.matmul(ps_den[p][0:NSEG, 0:F], lhsT,
                              e_sl(ebuf, ch, t, t + 1),
                              start=start, stop=stop,
                              skip_group_check=True)
                    mm = te.matmul(
                        ps_num[p][0:NSEG, 0:F], lhsT,
                        e_sl(exbuf, ch, t, t + 1),
                        start=start, stop=stop,
                        skip_group_check=True)
                    # same one-hot already resident in the PE array: skip
                    # the redundant LDWEIGHTS (halves PE instruction cost)
                    mm.ins.ldweights = False
                    if t == nt - 1:
                        mm.then_inc(s_mm, 1)
            te.drain().then_inc(s_pe_done, 1)
    return nc


def kernel(**inputs):
    global LAST_EXEC_TIME_NS
    from concourse.bass_utils import run_bass_kernel_spmd

    m = {"u": np.ascontiguousarray(inputs["m_u"], dtype=np.float32).reshape(-1, F),
         "v": np.ascontiguousarray(inputs["m_v"], dtype=np.float32).reshape(-1, F),
         "y": np.ascontiguousarray(inputs["m_y"], dtype=np.float32).reshape(-1, F)}
    idx = {p: np.asarray(inputs[f"batch_{p}"]).astype(np.int64) for p in "uvy"}
    t_vals = [float(np.asarray(inputs[f"t_{p}"]).reshape(-1)[0]) for p in "uvy"]
    W = np.asarray(inputs["W"], dtype=np.float32)
    bias = np.asarray(inputs["b"], dtype=np.float32)

    planes = ["u", "v", "y"]
    bounds = {p: np.searchsorted(idx[p], np.arange(B + 1), side="left")
              for p in planes}
    core_rng = {p: [(int(bounds[p][NSEG * k]), int(bounds[p][NSEG * (k + 1)]))
                    for k in range(N_CORES)] for p in planes}
    max_n = max(b - a for p in planes for (a, b) in core_rng[p])
    p_n = max(128, -(-max_n // 128) * 128)

    key = (p_n, tuple(t_vals))
    if key not in _prog_cache:
        _prog_cache[key] = _build_program(p_n, t_vals)
    nc = _prog_cache[key]

    chunks, total_tiles = _plan(p_n)
    CWF = E_OUT * 3 * F + E_OUT

    m_bf = {p: m[p].astype(BF16NP) for p in planes}
    seg_iota = np.arange(NSEG, dtype=np.float32)

    in_maps = []
    for k in range(N_CORES):
        cwf = np.zeros((128, CWF), np.float32)
        cwf[:NSEG, :E_OUT * 3 * F] = W.reshape(1, -1)
        cwf[:NSEG, E_OUT * 3 * F:] = bias
        d = {"constsW": cwf}
        for pi, p in enumerate(planes):
            a, b_ = core_rng[p][k]
            n = b_ - a
            xp = np.zeros((p_n, F), BF16NP)
            xp[:n] = m_bf[p][a:b_]
            ip = np.full((p_n,), PAD_SEG, np.float32)
            ip[:n] = (idx[p][a:b_] - NSEG * k).astype(np.float32)
            ohp = (ip[:, None] == seg_iota[None, :]).astype(BF16NP)
            # per-chunk contiguous block [128, nt*FO] (C-order): partition
            # p's chunk cols = [x tiles t-major | oh tiles t-major], stored
            # as DRAM rows p*nt + t of FO elems.  Matches x_sl/oh_sl.
            blocks = []
            for ch in chunks:
                if ch["plane"] != pi:
                    continue
                nt = ch["ntiles"]
                base = ch["g0"] * 128
                blk = np.empty((128, nt * FO), BF16NP)
                blk[:, :nt * F] = \
                    xp[base:base + nt * 128].reshape(nt, 128, F) \
                    .swapaxes(0, 1).reshape(128, nt * F)
                blk[:, nt * F:] = \
                    ohp[base:base + nt * 128].reshape(nt, 128, NSEG) \
                    .swapaxes(0, 1).reshape(128, nt * NSEG)
                blocks.append(blk.reshape(-1, FO))
            d[f"x{pi}"] = np.ascontiguousarray(np.concatenate(blocks, axis=0))
            assert d[f"x{pi}"].shape == (p_n, FO)
        in_maps.append(d)

    res = None
    last_err = None
    for _attempt in range(3):
        try:
            res = run_bass_kernel_spmd(nc, in_maps, list(range(N_CORES)))
            break
        except Exception as e:      # transient device faults: retry
            last_err = e
            import time as _time
            _time.sleep(2.0)
    if res is None:
        raise last_err
    LAST_EXEC_TIME_NS = res.exec_time_ns
    out = np.concatenate([res.results[k]["out"] for k in range(N_CORES)], axis=0)
    return out.astype(np.float32)
